# revision 1
# baseline (speedup 1.0000x reference)
"""DCNv2 deformable conv kernel for TRN2 (Bass/Tile), one image per core.

Layout (per core), honoring the SBUF start-partition rule (starts in {0,32,64,96}):
- channel-major activations [C=128 partitions, pixels], raster pixel order.
- padded image space: 60x60 (2-pixel zero border), flat idx = (y+2)*60 + (x+2).
- om2 [128, 3136] offset-conv output, quantity-per-quadrant rows:
    row 32u + 9g + t holds (u=0: off_y, u=1: off_x, u=2: mask-raw) for group g,
    tap t; och(u,g,t) = 18u + 9g + t. Rows 18..31 of each quadrant unused.
- gather: GPSIMD ap_gather of uint32 pairs (fp16 x[j] | fp16 x[j+1]) at clamped
  integer positions; gather-out col i <-> conv pixel c = 196*(i%16) + i//16.
- combine: v-pairs (via pi-inverse view) * DMA-broadcast interleaved
  weight-pairs; PE accumulates 4 neighbor streams x 9 taps into PSUM.
- BN: per-core (sum, sumsq) -> AllReduce -> affine apply.
"""
import numpy as np
import concourse.bass as bass
import concourse.mybir as mybir
import concourse.tile as tile
from concourse.masks import make_identity

AF = mybir.ActivationFunctionType
OP = mybir.AluOpType
F32 = mybir.dt.float32
F16 = mybir.dt.float16
I32 = mybir.dt.int32
I16 = mybir.dt.int16
U32 = mybir.dt.uint32

H = W = 56
HW = H * W            # 3136
PW = 60               # padded row stride
NPAD = 3712           # padded flat size (gather touches <= 3660)
NPADH = 3776          # xpadh fp16 size (pair build reads up to 3712)
NT = 9                # taps
NB = 8                # hw blocks
BLK = 392             # 7 rows of 56
HH = HW // 2          # column half
CHUNKH = HH // 16     # 98: idx-wrap chunk within a half


def host_constants(hw_round=True):
    """Input-independent constant tensors passed as extra kernel inputs.

    hw_round: the hardware float->int convert rounds to nearest-even, so the
    floor trick biases by +7.5 (RNE(v+7.5) = floor(v)+8). CoreSim truncates,
    where +8.0 gives the same result; pick per target."""
    bias = 7.5 if hw_round else 8.0
    baseC = np.zeros((128, HW), np.float32)
    hh, ww = np.meshgrid(np.arange(H), np.arange(W), indexing="ij")
    hh = hh.reshape(-1).astype(np.float32)
    ww = ww.reshape(-1).astype(np.float32)
    for g in range(2):
        for t in range(NT):
            baseC[0 + 9 * g + t] = hh - 1 + (t // 3) + bias
            baseC[32 + 9 * g + t] = ww - 1 + (t % 3) + bias
    s60 = np.zeros((128, 1), np.float32)
    s60b = np.zeros((128, 1), np.float32)
    s60[0:18] = 60.0
    s60[32:50] = 1.0
    s60b[0:18] = 6.0 * 60.0 - 183.0     # 177
    s60b[32:50] = 6.0 * 1.0 - 183.0     # -177
    return baseC, s60, s60b


def rap(t, part_off, part_step, part_cnt, free_off, free_dims):
    """Strided AP into tile/AP t. part_* in partition-rows, free in elements."""
    a = t if isinstance(t, bass.AP) else t[:]
    row = a.ap[0][0]
    return bass.AP(tensor=a.tensor, offset=a.offset + part_off * row + free_off,
                   ap=[[part_step * row, part_cnt]] + [list(d) for d in free_dims])


def emit(tc: tile.TileContext, outs, ins, num_cores: int, hw_round=True):
    nc = tc.nc
    (out_d,) = outs
    (x_d, woff_d, boff_d, w_d, b_d, gam_d, bet_d, baseC_d, s60_d, s60b_d) = ins

    norm = 1.0 / float(num_cores * HW)

    with tc.tile_pool(name="consts", bufs=1) as consts, \
         tc.tile_pool(name="dram", bufs=1, space="DRAM") as dram, \
         tc.tile_pool(name="main", bufs=1) as main:

        # ---------- small constant tiles ----------
        s60 = consts.tile([128, 1], F32); nc.sync.dma_start(out=s60[:], in_=s60_d[:])
        s60b = consts.tile([128, 1], F32); nc.sync.dma_start(out=s60b[:], in_=s60b_d[:])
        b_t = consts.tile([128, 1], F32); nc.sync.dma_start(out=b_t[:], in_=b_d[:])
        gam_t = consts.tile([128, 1], F32); nc.sync.dma_start(out=gam_t[:], in_=gam_d[:])
        bet_t = consts.tile([128, 1], F32); nc.sync.dma_start(out=bet_t[:], in_=bet_d[:])
        c65 = consts.tile([128, 1], F32); nc.vector.memset(c65[:], 65.0)
        c59 = consts.tile([128, 1], F32); nc.vector.memset(c59[:], 59.0)
        ident = consts.tile([128, 128], F32)
        make_identity(nc, ident[:])

        # b_off2 [128,1]: per om2-row bias = b_off[och(row)]
        b_off2 = consts.tile([128, 1], F32)
        nc.vector.memset(b_off2[:], 0.0)
        bofr = boff_d[:]
        for u in range(3):
            nc.sync.dma_start(
                out=rap(b_off2, 32 * u, 1, 18, 0, [[1, 1]]),
                in_=bass.AP(tensor=bofr.tensor, offset=bofr.offset + 18 * u,
                            ap=[[1, 18], [0, 1]]),
            )

        # ---------- weights prep ----------
        lhsT_off = consts.tile([128, NT * 128], F16)
        nc.vector.memset(lhsT_off[:], 0.0)
        wT = consts.tile([128, NT * 128], F16)

        with tc.tile_pool(name="wprep", bufs=1) as wprep, \
             tc.tile_pool(name="pswp", bufs=2, space="PSUM") as pswp:
            woff_sb = wprep.tile([54, 1152], F32)
            nc.sync.dma_start(out=woff_sb[:], in_=woff_d[:])
            w_sb = wprep.tile([128, 1152], F32)
            nc.sync.dma_start(out=w_sb[:], in_=w_d[:])
            for t in range(NT):
                tp1 = pswp.tile([128, 128], F32, tag="tp")
                nc.tensor.transpose(
                    tp1[:, :54], rap(woff_sb, 0, 1, 54, t, [[9, 128]]),
                    ident[:54, :54])
                nc.scalar.copy(
                    out=rap(lhsT_off, 0, 1, 128, 128 * t, [[32, 3], [1, 18]]),
                    in_=rap(tp1, 0, 1, 128, 0, [[18, 3], [1, 18]]),
                )
                tp2 = pswp.tile([128, 128], F32, tag="tp")
                nc.tensor.transpose(
                    tp2[:], rap(w_sb, 0, 1, 128, t, [[9, 128]]), ident[:])
                nc.scalar.copy(out=wT[:, 128 * t:128 * (t + 1)], in_=tp2[:])

        xpair = main.tile([128, NPAD], U32)

        wd = dram.tile([2, 18, 2 * HW], F16)      # [pair-class, (g,t), interleave]
        idxd = dram.tile([2, 18, HW], I16)        # [row01, (g,t), pixel]
        idxw = dram.tile([2, 4, 16 * 9 * CHUNKH], I16)  # [rp, (g,half), (s,t,j)]

        with tc.tile_pool(name="mid", bufs=1) as mid:
            # ---------- padded relu image (fp16) + pair tensor ----------
            xpadh = mid.tile([128, NPADH], F16)
            nc.vector.memset(xpadh[:], 0.0)
            xraw = mid.tile([128, HW], F16, tag="sA")
            nc.sync.dma_start(out=xraw[:], in_=x_d[:])
            nc.scalar.activation(
                rap(xpadh, 0, 1, 128, 2 * PW + 2, [[PW, H], [1, W]]),
                xraw[:], AF.Relu)
            xpair_h = xpair.bitcast(F16)
            nc.scalar.copy(out=rap(xpair_h, 0, 1, 128, 0, [[2, NPAD]]),
                           in_=rap(xpadh, 0, 1, 128, 0, [[1, NPAD]]))
            nc.scalar.copy(out=rap(xpair_h, 0, 1, 128, 1, [[2, NPAD]]),
                           in_=rap(xpadh, 0, 1, 128, 1, [[1, NPAD]]))

            # ---------- offset conv -> om2 ----------
            om2 = mid.tile([128, HW], F32, tag="sC")
            with tc.tile_pool(name="psconv", bufs=2, space="PSUM") as psconv:
                for blk in range(NB):
                    pom = psconv.tile([128, BLK], F32, tag="pom")
                    for t in range(NT):
                        ki, kj = t // 3, t % 3
                        rhs = rap(xpadh, 0, 1, 128,
                                  (7 * blk + ki + 1) * PW + kj + 1,
                                  [[PW, 7], [1, W]])
                        nc.tensor.matmul(pom[:],
                                         lhsT_off[:, 128 * t:128 * (t + 1)], rhs,
                                         start=(t == 0), stop=(t == NT - 1))
                    nc.scalar.activation(om2[:, BLK * blk:BLK * (blk + 1)], pom[:],
                                         AF.Identity, bias=b_off2[:])

            # ---------- pipeline (column-split to pipeline the chain) ----
            baseC = mid.tile([128, HW], F16, tag="sA")
            nc.sync.dma_start(out=baseC[:], in_=baseC_d[:])
            msk = mid.tile([128, HW], F32, tag="sD")
            yxi = mid.tile([128, HW], I32, tag="sB")
            yxf = mid.tile([128, HW], F32, tag="sE")
            wfrac = mid.tile([128, HW], F32, tag="sF")
            wx_a = mid.tile([18, HW], F32)
            m_a = mid.tile([18, HW], F32)
            xi_a = mid.tile([18, HW], F32)
            idxr0 = mid.tile([18, HW], F32, tag="sA")
            idxr1 = mid.tile([18, HW], F32, tag="sB")
            idx16a = mid.tile([18, HW], I16)
            idx16b = mid.tile([18, HW], I16)
            pA = mid.tile([18, HW], F32, tag="sA")
            ay0 = mid.tile([18, HW], F32, tag="sB")
            bx0 = mid.tile([18, HW], F32, tag="sC")
            wint0 = mid.tile([18, 2 * HW], F16, tag="sD")
            wint1 = mid.tile([18, 2 * HW], F16, tag="sE")
            for c0 in (0, HH):
                sl = slice(c0, c0 + HH)
                s2 = slice(2 * c0, 2 * (c0 + HH))
                nc.scalar.activation(msk[:, sl], om2[:, sl], AF.Sigmoid)
                nc.vector.tensor_tensor(out=om2[:, sl], in0=om2[:, sl],
                                        in1=baseC[:, sl], op=OP.add)
                nc.vector.tensor_copy(out=yxi[:, sl], in_=om2[:, sl])
                nc.vector.tensor_copy(out=yxf[:, sl], in_=yxi[:, sl])
                nc.vector.scalar_tensor_tensor(
                    out=wfrac[:, sl], in0=om2[:, sl],
                    scalar=0.5 if hw_round else 0.0,
                    in1=yxf[:, sl], op0=OP.add, op1=OP.subtract)
                nc.scalar.activation(yxf[:, sl], yxf[:, sl], AF.Relu,
                                     bias=c65[:], scale=-1.0)
                nc.scalar.activation(yxf[:, sl], yxf[:, sl], AF.Relu,
                                     bias=c59[:], scale=-1.0)
                nc.scalar.activation(yxf[:, sl], yxf[:, sl], AF.Identity,
                                     bias=s60b[:], scale=s60[:])
                nc.sync.dma_start(out=wx_a[:, sl], in_=wfrac[32:50, sl])
                nc.sync.dma_start(out=m_a[:, sl], in_=msk[64:82, sl])
                nc.sync.dma_start(out=xi_a[:, sl], in_=yxf[32:50, sl])
                nc.vector.tensor_tensor(out=idxr0[:, sl], in0=yxf[0:18, sl],
                                        in1=xi_a[:, sl], op=OP.add)
                nc.vector.tensor_scalar(out=idxr1[:, sl], in0=idxr0[:, sl],
                                        scalar1=60.0, scalar2=None, op0=OP.add)
                nc.vector.tensor_copy(out=idx16a[:, sl], in_=idxr0[:, sl])
                nc.vector.tensor_copy(out=idx16b[:, sl], in_=idxr1[:, sl])
                nc.sync.dma_start(
                    out=rap(idxd[0], 0, 1, 18, c0, [[1, HH]]),
                    in_=idx16a[:, sl])
                nc.sync.dma_start(
                    out=rap(idxd[1], 0, 1, 18, c0, [[1, HH]]),
                    in_=idx16b[:, sl])
                nc.vector.tensor_tensor(out=pA[:, sl], in0=wfrac[0:18, sl],
                                        in1=m_a[:, sl], op=OP.mult)
                nc.vector.tensor_tensor(out=ay0[:, sl], in0=m_a[:, sl],
                                        in1=pA[:, sl], op=OP.subtract)
                nc.scalar.activation(bx0[:, sl], wx_a[:, sl], AF.Identity,
                                     bias=1.0, scale=-1.0)
                nc.vector.tensor_tensor(
                    out=rap(wint0, 0, 1, 18, 2 * c0, [[2, HH]]),
                    in0=ay0[:, sl], in1=bx0[:, sl], op=OP.mult)
                nc.vector.tensor_tensor(
                    out=rap(wint0, 0, 1, 18, 2 * c0 + 1, [[2, HH]]),
                    in0=ay0[:, sl], in1=wx_a[:, sl], op=OP.mult)
                nc.vector.tensor_tensor(
                    out=rap(wint1, 0, 1, 18, 2 * c0, [[2, HH]]),
                    in0=pA[:, sl], in1=bx0[:, sl], op=OP.mult)
                nc.vector.tensor_tensor(
                    out=rap(wint1, 0, 1, 18, 2 * c0 + 1, [[2, HH]]),
                    in0=pA[:, sl], in1=wx_a[:, sl], op=OP.mult)
                nc.sync.dma_start(
                    out=rap(wd[0], 0, 1, 18, s2.start, [[1, 2 * HH]]),
                    in_=wint0[:, s2])
                nc.sync.dma_start(
                    out=rap(wd[1], 0, 1, 18, s2.start, [[1, 2 * HH]]),
                    in_=wint1[:, s2])

        # ---------- idx wrap (DRAM->DRAM) + idxT, per column half ----------
        for rp in range(2):
            src = idxd[rp]
            for gh in range(2):
                for hf in range(2):
                    nc.sync.dma_start(
                        out=idxw[rp, 2 * gh + hf],
                        in_=bass.AP(tensor=src.tensor,
                                    offset=src.offset + 9 * gh * HW + hf * HH,
                                    ap=[[CHUNKH, 16], [HW, 9], [1, CHUNKH]]),
                    )
        # idxT free layout: [(half, tap) blocks of 196][rp * 98 + j]
        idxT = main.tile([128, 36 * CHUNKH], I16)
        for rp in range(2):
            for gh in range(2):
                for hf in range(2):
                    src = idxw[rp, 2 * gh + hf]
                    nc.sync.dma_start(
                        out=rap(idxT, 64 * gh, 1, 64,
                                (9 * hf) * 2 * CHUNKH + rp * CHUNKH,
                                [[2 * CHUNKH, 9], [1, CHUNKH]]),
                        in_=bass.AP(tensor=src.tensor, offset=src.offset,
                                    ap=[[0, 4], [1, 16 * 9 * CHUNKH]]),
                    )

        # ---------- main loop over taps ----------
        outsb = main.tile([128, HW], F32)
        bsum = main.tile([128, NB], F32)
        with tc.tile_pool(name="psmain", bufs=1, space="PSUM") as psmain, \
             tc.tile_pool(name="taps", bufs=2) as taps, \
             tc.tile_pool(name="tapsv", bufs=2) as tapsv:
            psum_out = [psmain.tile([128, BLK], F32, tag=f"po{b}",
                                    name=f"po{b}") for b in range(NB)]
            for t in range(NT):
                for hf in range(2):
                    gt = taps.tile([128, 2 * HH], U32, tag=f"gt{hf}",
                                   name=f"gt{hf}")
                    bi = 9 * hf + t
                    nc.gpsimd.ap_gather(
                        gt[:], xpair[:],
                        idxT[:, bi * 2 * CHUNKH:(bi + 1) * 2 * CHUNKH],
                        channels=128, num_elems=NPAD, d=1, num_idxs=2 * HH)
                    wb = taps.tile([128, 4 * HH], F16, tag=f"wb{hf}",
                                   name=f"wb{hf}")
                    for gh in range(2):
                        for pc in range(2):
                            src = wd[pc]
                            nc.sync.dma_start(
                                out=rap(wb, 64 * gh, 1, 64, pc * 2 * HH,
                                        [[1, 2 * HH]]),
                                in_=bass.AP(
                                    tensor=src.tensor,
                                    offset=(src.offset
                                            + (9 * gh + t) * 2 * HW
                                            + hf * 2 * HH),
                                    ap=[[0, 64], [1, 2 * HH]]))
                    vt = tapsv.tile([128, 4 * HH], F16, tag=f"vt{hf}",
                                    name=f"vt{hf}")
                    gth = gt.bitcast(F16)
                    for rp in range(2):
                        gview = rap(gth, 0, 1, 128, rp * 2 * HH,
                                    [[2, 16], [32, CHUNKH], [1, 2]])
                        nc.vector.tensor_tensor(
                            out=vt[:, rp * 2 * HH:(rp + 1) * 2 * HH],
                            in0=gview, in1=wb[:, rp * 2 * HH:(rp + 1) * 2 * HH],
                            op=OP.mult)
                    for bl in range(NB // 2):
                        blk = 4 * hf + bl
                        for rp in range(2):
                            for par in range(2):
                                rhs = rap(vt, 0, 1, 128,
                                          rp * 2 * HH + 2 * BLK * bl + par,
                                          [[2, BLK]])
                                nc.tensor.matmul(
                                    psum_out[blk][:],
                                    wT[:, 128 * t:128 * (t + 1)], rhs,
                                    start=(t == 0 and rp == 0 and par == 0),
                                    stop=(t == NT - 1 and rp == 1
                                          and par == 1),
                                    skip_group_check=True)

            # ---------- bias + stats ----------
            for blk in range(NB):
                nc.scalar.activation(outsb[:, BLK * blk:BLK * (blk + 1)],
                                     psum_out[blk][:], AF.Identity, bias=b_t[:],
                                     accum_out=bsum[:, blk:blk + 1])

        sqd = main.tile([128, HW], F16)
        sacc = main.tile([128, 1], F32)
        nc.scalar.activation(sqd[:], outsb[:], AF.Square, accum_out=sacc[:])
        stats = main.tile([128, 2], F32)
        dump7 = main.tile([128, NB], F32)
        nc.scalar.activation(dump7[:], bsum[:], AF.Identity,
                             accum_out=stats[:, 0:1])
        nc.vector.tensor_copy(out=stats[:, 1:2], in_=sacc[:])

        # ---------- all-reduce stats ----------
        statd = dram.tile([128, 2], F32)
        statr = dram.tile([128, 2], F32)
        nc.sync.dma_start(out=statd[:], in_=stats[:])
        if num_cores > 1:
            nc.gpsimd.collective_compute(
                "AllReduce", OP.add,
                replica_groups=[list(range(num_cores))],
                ins=[statd.opt()], outs=[statr.opt()])
        else:
            nc.sync.dma_start(out=statr[:], in_=statd[:])
        st = main.tile([128, 2], F32)
        nc.sync.dma_start(out=st[:], in_=statr[:])

        # ---------- finalize BN ----------
        nc.vector.tensor_scalar(out=st[:], in0=st[:], scalar1=norm,
                                scalar2=None, op0=OP.mult)
        var = main.tile([128, 1], F32)
        nc.vector.tensor_tensor(out=var[:], in0=st[:, 0:1], in1=st[:, 0:1],
                                op=OP.mult)
        nc.vector.tensor_tensor(out=var[:], in0=st[:, 1:2], in1=var[:],
                                op=OP.subtract)
        nc.vector.tensor_scalar(out=var[:], in0=var[:], scalar1=1e-5,
                                scalar2=None, op0=OP.add)
        sd = main.tile([128, 1], F32)
        nc.scalar.activation(sd[:], var[:], AF.Sqrt)
        rs = main.tile([128, 1], F32)
        nc.vector.reciprocal(rs[:], sd[:])
        scl = main.tile([128, 1], F32)
        nc.vector.tensor_tensor(out=scl[:], in0=rs[:], in1=gam_t[:], op=OP.mult)
        bnb = main.tile([128, 1], F32)
        nc.vector.tensor_tensor(out=bnb[:], in0=st[:, 0:1], in1=scl[:],
                                op=OP.mult)
        nc.vector.tensor_tensor(out=bnb[:], in0=bet_t[:], in1=bnb[:],
                                op=OP.subtract)
        for c0 in (0, HW // 2):
            sl = slice(c0, c0 + HW // 2)
            nc.scalar.activation(outsb[:, sl], outsb[:, sl], AF.Identity,
                                 bias=bnb[:], scale=scl[:])
            nc.sync.dma_start(out=rap(out_d, 0, 1, 128, c0, [[1, HW // 2]]),
                              in_=outsb[:, sl])




# ----------------------------------------------------------------------------
# Host-side runner: shard batch over 8 cores, compile once, execute via SPMD.
# ----------------------------------------------------------------------------
import concourse.bacc as bacc
from concourse.bass_utils import run_bass_kernel_spmd

B = 8
N_CORES = 8

_CACHE = {}


def _build(num_cores=N_CORES):
    key = ("nc", num_cores)
    if key in _CACHE:
        return _CACHE[key]
    nc = bacc.Bacc("TRN2", target_bir_lowering=False, debug=False,
                   enable_asserts=False, num_devices=num_cores)
    x_d = nc.dram_tensor("x", [128, HW], F16, kind="ExternalInput").ap()
    woff_d = nc.dram_tensor("woff", [54, 1152], F32, kind="ExternalInput").ap()
    boff_d = nc.dram_tensor("boff", [54], F32, kind="ExternalInput").ap()
    w_d = nc.dram_tensor("w", [128, 1152], F32, kind="ExternalInput").ap()
    b_d = nc.dram_tensor("b", [128, 1], F32, kind="ExternalInput").ap()
    gam_d = nc.dram_tensor("gam", [128, 1], F32, kind="ExternalInput").ap()
    bet_d = nc.dram_tensor("bet", [128, 1], F32, kind="ExternalInput").ap()
    baseC_d = nc.dram_tensor("baseC", [128, HW], F16, kind="ExternalInput").ap()
    s60_d = nc.dram_tensor("s60", [128, 1], F32, kind="ExternalInput").ap()
    s60b_d = nc.dram_tensor("s60b", [128, 1], F32, kind="ExternalInput").ap()
    out_d = nc.dram_tensor("out", [128, HW], F32, kind="ExternalOutput").ap()
    with tile.TileContext(nc) as tc:
        emit(tc, [out_d],
             [x_d, woff_d, boff_d, w_d, b_d, gam_d, bet_d, baseC_d, s60_d,
              s60b_d], num_cores)
    nc.compile()
    _CACHE[key] = nc
    return nc


def _in_maps(inputs, num_cores=N_CORES):
    x = np.asarray(inputs["x"], np.float32)
    baseC, s60, s60b = host_constants()
    baseC = baseC.astype(np.float16)
    base = {
        "woff": np.ascontiguousarray(np.asarray(inputs["w_off"], np.float32)
                                     .reshape(54, 1152)),
        "boff": np.ascontiguousarray(np.asarray(inputs["b_off"], np.float32)
                                     .reshape(54)),
        "w": np.ascontiguousarray(np.asarray(inputs["w"], np.float32)
                                  .reshape(128, 1152)),
        "b": np.ascontiguousarray(np.asarray(inputs["b"], np.float32)
                                  .reshape(128, 1)),
        "gam": np.ascontiguousarray(np.asarray(inputs["gamma"], np.float32)
                                    .reshape(128, 1)),
        "bet": np.ascontiguousarray(np.asarray(inputs["beta"], np.float32)
                                    .reshape(128, 1)),
        "baseC": baseC, "s60": s60, "s60b": s60b,
    }
    return [dict(base, x=np.ascontiguousarray(x[c].reshape(128, HW)).astype(np.float16))
            for c in range(num_cores)]


def run(inputs, trace=False, **kw):
    """Run the SPMD kernel; returns (output [8,128,56,56], BassKernelResults)."""
    nc = _build(N_CORES)
    res = run_bass_kernel_spmd(nc, _in_maps(inputs), list(range(N_CORES)),
                               trace=trace, **kw)
    out = np.stack([res.results[c]["out"].reshape(128, H, W)
                    for c in range(N_CORES)]).astype(np.float32)
    return out, res


def kernel(**inputs) -> np.ndarray:
    out, _ = run(inputs, trace=False)
    return out

